# revision 34
# baseline (speedup 1.0000x reference)
"""Trainium2 Bass kernel for nn_ChangepointDetector.

Model (B=32, S=2048, I=32, W=20, H=128):
  win[t]  = x[t:t+20, :] flattened                      (sliding windows)
  h       = win @ W_enc + b_enc                         (B, nwin=2029, 128)
  enc     = gelu(LayerNorm(h) * gamma + beta)
  z1      = gelu([enc[t], enc[t+20]] @ W1 + b1)         (t in [0, T=2008))
  z2      = gelu(z1 @ W2 + b2)
  p       = sigmoid(z2 @ W3 + b3)                       -> pad to (B, S)

Sharding: pure data parallel, 4 batches per core across 8 cores.

Device kernel design (per core, channels-on-partitions layout):
  * Host passes x transposed per batch (xT [32, 2048]) so the device can
    build X4 [128, 2045] (4 shift-stacked copies of xT: X4[32j+i, s] =
    x[s+j, i]) with big-descriptor DMAs.  The encoder contraction
    (K = 20*32 = 640) then becomes 5 dense K=128 matmuls per window tile,
    with the rhs being plain offset views into X4 (no im2col blowup).
  * The encoder runs in split-precision fp16: x and W_enc are decomposed
    on the host into hi + lo fp16 halves (x = xh + xl exactly to ~22
    mantissa bits), and x.W is computed as xh.Wh + xl.Wh + xh.Wl - three
    1-cycle/column fp16 matmuls accumulated in fp32 PSUM, ~1.5x faster
    than native fp32 (4 cycles/column) at near-fp32 accuracy (verified
    on HW: 2.5e-7 vs 1.8e-7 relative).  The comparator stays fp32.
  * W_enc/b_enc are mean-centered over H on the host, which makes the
    LayerNorm mean-subtraction exact and free (h comes out of the GEMM
    already centered).
  * LN variance: DVE square + ones-matmul (partition reduction; outputs
    col-packed to PSUM rows {0,32,64,96}, gathered to dense rows by DMA),
    rstd via a table-free Newton rsqrt on the vector engine (var is
    ~1 +- 0.5 so a linear seed + 3 NR iterations reaches fp32 accuracy).
    This keeps the ACT engine on a single table set (gelu) all kernel.
  * rstd goes through a small DRAM bounce so a partition-step-0 DMA can
    broadcast it across partitions (keeps the PE free of broadcast MMs).
  * The comparator concat is just two offset views of enc (t and t+20),
    so L1 is 2 accumulating matmuls; L2/L3 are single matmuls.  The four
    L3 outputs (M=1) of a batch land on PSUM partitions {0,32,64,96} of
    one bank via tile_position col packing.
  * Emission is phase-split (all encoders -> all normalizes -> all
    comparators) so cross-batch work hides the stats/broadcast chains,
    and a few warmup matmuls on constant data ramp the PE clock gate
    while the first input DMAs are in flight.
  * Device returns pre-sigmoid logits; sigmoid + b3 + padding + threshold
    run on the host (monotonic, so probs > 0.5 matches z3 + b3 > 0).

The reference's probs concentrate near 0.5, so the boolean output cannot
survive genuinely low-precision matmuls (bf16 ~1e-2, float32r ~2e-4 were
measured and rejected); the fp16 hi/lo split keeps full fp32-class
accuracy (zero boolean flips measured on the grading inputs).
"""

import os
import numpy as np

# ---------------------------------------------------------------- constants
B, S, I, W, H = 32, 2048, 32, 20, 128
NWIN = S - W + 1          # 2029
T = S - 2 * W             # 2008
NCORES = 8
NB = B // NCORES          # 4 batches per core
KT = (W * I) // 128       # 5 k-tiles of 128
TN = [512, 512, 512, NWIN - 3 * 512]   # encoder window tiles (last 493)
CN = [512, 512, 512, T - 3 * 512]      # comparator tiles (last 472)
X4_COLS = NWIN + 4 * (KT - 1)          # 2045 columns of X4 actually used
LN_EPS = 1e-5

# Newton rsqrt seed: least-squares linear fit of v**-0.5 on [0.3, 2.5]
# (relative-error weighted).  4 NR iterations afterwards reach ~1e-8.
_vs = np.linspace(0.40, 1.85, 4001)
_w = _vs ** -0.25
_SEED_B, _SEED_A = np.polyfit(_vs, _vs ** -0.5, 1, w=_w)
NR_ITERS = 3

_BUILT = {}


def _build_nc():
    """Build + compile the single-core Bass program (same on all 8 cores)."""
    import concourse.bass as bass
    import concourse.tile as tile
    from concourse import bacc, mybir

    f32 = mybir.dt.float32
    AF = mybir.ActivationFunctionType
    OP = mybir.AluOpType

    nc = bacc.Bacc(
        "TRN2",
        target_bir_lowering=False,
        debug=False,
        enable_asserts=True,
        num_devices=NCORES,
    )

    f16 = mybir.dt.float16
    xth = nc.dram_tensor("xth", [NB, 32, S], f16, kind="ExternalInput").ap()
    xtl = nc.dram_tensor("xtl", [NB, 32, S], f16, kind="ExternalInput").ap()
    wench = nc.dram_tensor("wench", [128, KT, 128], f16, kind="ExternalInput").ap()
    wencl = nc.dram_tensor("wencl", [128, KT, 128], f16, kind="ExternalInput").ap()
    w1 = nc.dram_tensor("w1", [128, 2, 128], f32, kind="ExternalInput").ap()
    w2 = nc.dram_tensor("w2", [128, 64], f32, kind="ExternalInput").ap()
    w3 = nc.dram_tensor("w3", [64, 1], f32, kind="ExternalInput").ap()
    vecs = nc.dram_tensor("vecs", [128, 8], f32, kind="ExternalInput").ap()
    out = nc.dram_tensor("out", [NB, 4, 512], f32, kind="ExternalOutput").ap()

    def srows(t, n=512):
        # rows {0,32,64,96} of a [128, n] tile as a [4, n] strided AP
        return t.rearrange("(a b) n -> a b n", b=32)[:, 0, 0:n]

    from contextlib import ExitStack

    with tile.TileContext(nc) as tc, ExitStack() as ctx:
        consts = ctx.enter_context(tc.tile_pool(name="consts", bufs=1))
        x4p = ctx.enter_context(tc.tile_pool(name="x4p", bufs=2))
        hp = ctx.enter_context(tc.tile_pool(name="hp", bufs=4))
        encp = ctx.enter_context(tc.tile_pool(name="encp", bufs=4))
        wrk = ctx.enter_context(tc.tile_pool(name="wrk", bufs=3))
        nrp = ctx.enter_context(tc.tile_pool(name="nrp", bufs=3))
        lgp = ctx.enter_context(tc.tile_pool(name="lgp", bufs=2))
        php = ctx.enter_context(tc.tile_pool(name="php", bufs=2, space="PSUM"))
        psp = ctx.enter_context(tc.tile_pool(name="psp", bufs=1, space="PSUM"))
        pz1p = ctx.enter_context(tc.tile_pool(name="pz1p", bufs=2, space="PSUM"))
        pz2p = ctx.enter_context(tc.tile_pool(name="pz2p", bufs=2, space="PSUM"))
        pz3p = ctx.enter_context(tc.tile_pool(name="pz3p", bufs=1, space="PSUM"))
        prsb = ctx.enter_context(tc.tile_pool(name="prsb", bufs=3))
        drp = ctx.enter_context(tc.tile_pool(name="drp", bufs=4, space="DRAM"))

        whi_sb = consts.tile([128, KT, 128], f16, tag="wench")
        for kt in range(KT):
            nc.sync.dma_start(out=whi_sb[:, kt, :], in_=wench[:, kt, :])
        wlo_sb = consts.tile([128, KT, 128], f16, tag="wencl")
        w1_sb = consts.tile([128, 2, 128], f32, tag="w1")
        w2_sb = consts.tile([128, 64], f32, tag="w2")
        w3_sb = consts.tile([64, 1], f32, tag="w3")
        vecs_sb = consts.tile([128, 8], f32, tag="vecs")
        ones_sb = consts.tile([128, 128], f32, tag="ones")
        nc.vector.memset(ones_sb[:, :], 1.0)

        def _late_consts():
            nc.sync.dma_start(out=wlo_sb[:, :, :], in_=wencl)
            nc.sync.dma_start(out=w1_sb[:, :, :], in_=w1)
            nc.sync.dma_start(out=w2_sb[:, :], in_=w2)
            nc.sync.dma_start(out=w3_sb[:, :], in_=w3)
            nc.sync.dma_start(out=vecs_sb[:, :], in_=vecs)

        bc_col = vecs_sb[:, 0:1]      # centered encoder bias
        gamma_col = vecs_sb[:, 1:2]
        beta_col = vecs_sb[:, 2:3]
        b1_col = vecs_sb[:, 3:4]
        b2_col = vecs_sb[0:64, 4:5]

        # PE warmup: matmuls on constant data (no DMA dependency) ramp the
        # HAM clock gate to 8/8 while the first input DMAs are in flight.
        pwarm = php.tile([128, 512], f32, tag="ph")
        for _ in range(6):
            nc.tensor.matmul(
                pwarm[:, 0:128], lhsT=ones_sb[:, :], rhs=ones_sb[:, :],
                start=True, stop=True,
            )

        rds, hs, encs = [], [], []
        for b in range(NB):
            # ---- input: build X4 (4 shifted copies of xT) --------------
            x4h = x4p.tile([128, S], f16, tag="x4h")
            x4l = x4p.tile([128, S], f16, tag="x4l")
            for c0, c1 in ((0, 544), (544, 1056), (1056, X4_COLS)):
                for jj in range(4):
                    nc.scalar.dma_start(
                        out=x4h[32 * jj : 32 * jj + 32, c0:c1],
                        in_=xth[b, :, jj + c0 : jj + c1],
                    )
            for c0, c1 in ((0, 544), (544, 1056), (1056, X4_COLS)):
                for jj in range(4):
                    nc.sync.dma_start(
                        out=x4l[32 * jj : 32 * jj + 32, c0:c1],
                        in_=xtl[b, :, jj + c0 : jj + c1],
                    )
                if b == 0 and c0 == 0:
                    _late_consts()

            # ---- encoder GEMM + LN stats per window tile ---------------
            h = hp.tile([128, S], f32, tag="h")
            nc.vector.memset(h[:, NWIN:S], 0.0)
            ps = psp.tile([128, 512], f32, tag="ps")
            nc.vector.memset(ps[:, :], 0.0)
            for j in range(4):
                n, t0 = TN[j], 512 * j
                ph = php.tile([128, 512], f32, tag="ph")
                terms = []
                for kt in range(KT):
                    terms.append((whi_sb[:, kt, :], x4h, kt))
                for kt in range(KT):
                    terms.append((whi_sb[:, kt, :], x4l, kt))
                for kt in range(KT):
                    terms.append((wlo_sb[:, kt, :], x4h, kt))
                for i, (wt, xs, kt) in enumerate(terms):
                    nc.tensor.matmul(
                        ph[:, 0:n],
                        lhsT=wt,
                        rhs=xs[:, t0 + 4 * kt : t0 + 4 * kt + n],
                        start=(i == 0),
                        stop=(i == len(terms) - 1),
                    )
                nc.vector.tensor_scalar_add(
                    out=h[:, t0 : t0 + n], in0=ph[:, 0:n], scalar1=bc_col
                )
                sq = wrk.tile([128, 512], f32, tag="sq")
                nc.vector.tensor_mul(
                    out=sq[:, :], in0=h[:, t0 : t0 + 512], in1=h[:, t0 : t0 + 512]
                )
                nc.tensor.matmul(
                    ps[32 * j : 32 * j + 1, 0:512],
                    lhsT=ones_sb[:, 0:1],
                    rhs=sq[:, :],
                    start=True,
                    stop=True,
                    tile_position=(0, 32 * j),
                )

            # ---- rstd = (var + eps)**-0.5 via DVE Newton ---------------
            # gather psum rows {0,32,64,96} to dense partitions 0-3 by a
            # small matmul (compute engines cannot address strided rows)
            s_sb = nrp.tile([128, 512], f32, tag="s_sb")
            nc.vector.tensor_copy(out=s_sb[:, :], in_=ps[:, :])
            g4 = nrp.tile([4, 512], f32, tag="g4")
            nc.sync.dma_start(out=g4[:, :], in_=srows(s_sb))
            v = nrp.tile([4, 512], f32, tag="v")
            nc.vector.tensor_scalar(
                out=v[:, :], in0=g4[:, :], scalar1=1.0 / H, scalar2=LN_EPS,
                op0=OP.mult, op1=OP.add,
            )
            ya = nrp.tile([4, 512], f32, tag="ya")
            yb = nrp.tile([4, 512], f32, tag="yb")
            nc.vector.tensor_scalar(
                out=ya[:, :], in0=v[:, :], scalar1=float(_SEED_B),
                scalar2=float(_SEED_A), op0=OP.mult, op1=OP.add,
            )
            ycur, ynxt = ya, yb
            for _ in range(NR_ITERS):
                y2 = nrp.tile([4, 512], f32, tag="y2")
                nc.vector.tensor_mul(out=y2[:, :], in0=ycur[:, :], in1=ycur[:, :])
                nc.vector.tensor_mul(out=y2[:, :], in0=y2[:, :], in1=v[:, :])
                nc.vector.tensor_scalar(
                    out=y2[:, :], in0=y2[:, :], scalar1=-0.5, scalar2=1.5,
                    op0=OP.mult, op1=OP.add,
                )
                nc.vector.tensor_mul(out=ynxt[:, :], in0=ycur[:, :], in1=y2[:, :])
                ycur, ynxt = ynxt, ycur
            rstd = ycur
            # rstd -> DRAM so it can be partition-broadcast by DMA
            rd = drp.tile([4, 512], f32, tag="rd")
            rds.append(rd)
            hs.append(h)
            nc.sync.dma_start(out=rd[:, :], in_=rstd[:, :])


        for b in range(NB):
            h = hs[b]
            rd = rds[b]
            # ---- normalize + gelu -> enc -------------------------------
            enc = encp.tile([128, S], f32, tag="enc")
            encs.append(enc)
            for j in range(4):
                n, t0 = TN[j], 512 * j
                pr = prsb.tile([128, 512], f32, tag="pr")
                row = rd[j : j + 1, 0:n]
                row_bcast = bass.AP(
                    tensor=row.tensor, offset=row.offset,
                    ap=[[0, 128]] + [list(d) for d in row.ap[1:]],
                )
                nc.gpsimd.dma_start(out=pr[:, 0:n], in_=row_bcast)
                pre = wrk.tile([128, 512], f32, tag="pre")
                nc.vector.scalar_tensor_tensor(
                    out=pre[:, 0:n], in0=h[:, t0 : t0 + n], scalar=gamma_col,
                    in1=pr[:, 0:n], op0=OP.mult, op1=OP.mult,
                )
                nc.scalar.activation(
                    out=enc[:, t0 : t0 + n], in_=pre[:, 0:n], func=AF.Gelu,
                    bias=beta_col, scale=1.0,
                )


        for b in range(NB):
            enc = encs[b]
            # ---- comparator MLP ----------------------------------------
            pz3 = pz3p.tile([128, 512], f32, tag="pz3")
            nc.vector.memset(pz3[:, :], 0.0)
            for j in range(4):
                n, t0 = CN[j], 512 * j
                pz1 = pz1p.tile([128, 512], f32, tag="pz1")
                nc.tensor.matmul(
                    pz1[:, 0:n], lhsT=w1_sb[:, 0, :], rhs=enc[:, t0 : t0 + n],
                    start=True, stop=False,
                )
                nc.tensor.matmul(
                    pz1[:, 0:n], lhsT=w1_sb[:, 1, :], rhs=enc[:, t0 + W : t0 + W + n],
                    start=False, stop=True,
                )
                z1 = wrk.tile([128, 512], f32, tag="z1")
                nc.scalar.activation(
                    out=z1[:, 0:n], in_=pz1[:, 0:n], func=AF.Gelu,
                    bias=b1_col, scale=1.0,
                )
                pz2 = pz2p.tile([64, 512], f32, tag="pz2")
                nc.tensor.matmul(
                    pz2[:, 0:n], lhsT=w2_sb[:, :], rhs=z1[:, 0:n],
                    start=True, stop=True,
                )
                z2 = wrk.tile([64, 512], f32, tag="z2")
                nc.scalar.activation(
                    out=z2[:, 0:n], in_=pz2[:, 0:n], func=AF.Gelu,
                    bias=b2_col, scale=1.0,
                )
                nc.tensor.matmul(
                    pz3[32 * j : 32 * j + 1, 0:n], lhsT=w3_sb[:, :],
                    rhs=z2[0:64, 0:n], start=True, stop=True,
                    tile_position=(0, 32 * j),
                )

            lg = lgp.tile([128, 512], f32, tag="lg")
            nc.vector.tensor_copy(out=lg[:, :], in_=pz3[:, :])
            nc.sync.dma_start(out=out[b], in_=srows(lg))

    nc.compile()
    return nc


def _get_nc():
    if "nc" not in _BUILT:
        _BUILT["nc"] = _build_nc()
    return _BUILT["nc"]


def make_in_maps(x, W_enc, b_enc, gamma, beta, W1, b1, W2, b2, W3, b3):
    """Host-side prep: shard x, center the encoder weights, pack vectors."""
    x = np.ascontiguousarray(np.asarray(x, np.float32))
    W_enc = np.asarray(W_enc, np.float32)
    b_enc = np.asarray(b_enc, np.float32)

    W_c = W_enc - W_enc.mean(axis=1, keepdims=True)
    b_c = b_enc - b_enc.mean()
    wct = W_c.reshape(KT, 128, 128).transpose(1, 0, 2)
    wench = np.ascontiguousarray(wct.astype(np.float16))
    wencl = np.ascontiguousarray(
        (wct - wench.astype(np.float32)).astype(np.float16)
    )
    w1 = np.ascontiguousarray(
        np.asarray(W1, np.float32).reshape(2, 128, 128).transpose(1, 0, 2)
    )
    w2 = np.ascontiguousarray(np.asarray(W2, np.float32))
    w3 = np.ascontiguousarray(np.asarray(W3, np.float32).reshape(64, 1))
    vecs = np.zeros((128, 8), np.float32)
    vecs[:, 0] = b_c
    vecs[:, 1] = np.asarray(gamma, np.float32)
    vecs[:, 2] = np.asarray(beta, np.float32)
    vecs[:, 3] = np.asarray(b1, np.float32)
    vecs[0:64, 4] = np.asarray(b2, np.float32)

    xT = np.ascontiguousarray(x.transpose(0, 2, 1))  # [B, 32, S]
    xTh = xT.astype(np.float16)
    xTl = (xT - xTh.astype(np.float32)).astype(np.float16)
    in_maps = []
    for c in range(NCORES):
        sl = slice(NB * c, NB * (c + 1))
        in_maps.append(
            dict(
                xth=np.ascontiguousarray(xTh[sl]),
                xtl=np.ascontiguousarray(xTl[sl]),
                wench=wench, wencl=wencl, w1=w1, w2=w2, w3=w3, vecs=vecs,
            )
        )
    return in_maps


def assemble_output(core_outs, b3):
    """core_outs: list of 8 arrays [NB, 4, 512] of pre-b3 logits."""
    b3 = float(np.asarray(b3).reshape(-1)[0])
    logits = np.zeros((B, T), np.float32)
    for c, o in enumerate(core_outs):
        for bb in range(NB):
            row = []
            for j in range(4):
                row.append(o[bb, j, 0 : CN[j]])
            logits[NB * c + bb] = np.concatenate(row)
    z = (logits + b3).astype(np.float32)
    p = (1.0 / (1.0 + np.exp(-z.astype(np.float64)))).astype(np.float32)
    probs = np.zeros((B, S), np.float32)
    probs[:, W : W + T] = p
    return probs, probs > 0.5


def kernel(**inputs):
    from concourse.bass_utils import run_bass_kernel_spmd

    nc = _get_nc()
    in_maps = make_in_maps(**inputs)
    res = run_bass_kernel_spmd(nc, in_maps, core_ids=list(range(NCORES)))
    core_outs = [res.results[c]["out"] for c in range(NCORES)]
    return assemble_output(core_outs, inputs["b3"])
